# revision 10
# baseline (speedup 1.0000x reference)
# Multi-head attention kernel for Trainium2 (Bass/Tile), 8-core SPMD.
#
# Problem: B=4, S=2048, D=1024, H=16 heads, d_k=64 (fp32 in/out).
#
# Sharding: core c = (batch b, head-group g) with b = c//2, g = c%2.
# Each core computes 8 heads of one batch entirely on-device and emits the
# partial final projection (out_heads @ Wo_slice^T) over the full model dim.
# Host sums the two partial outputs per batch and adds the linear bias terms.
#
# Engine budget per core (bf16, warm 2.4 GHz):
#   scores   512 MMs (64-contr, h0/h1 row-group PAIRED)  ~55-65us
#   attn-V   512 MMs (128-contr, N=512)                  ~110us
#   proj     K/Q/V 384 MMs + outproj 128 MMs             ~111us
#   exp      171 ACTIVATE groups of FD=1536              ~262us ScalarE
# PE ~285us busy vs ScalarE ~262us -> PE is critical; the schedule keeps
# the PE dense and starts the exp stream as early as possible (~11us).
#
# Emission structure (engine order == emission order):
#   ramp:   warmup burst (HAM un-throttle) while kw/qw/x(sc0,sc1) stream in;
#           then ONLY K(sc0,j0), K(sc1,j0), Q(qc0,j0) inline -> first score
#           pair ~24 MMs in.  All other projections become bg items, ordered
#           so each is pumped a few pairs before its first consumer.
#   pairs:  per (qc,hp) block, 16 kc-pairs; the two MMs of a pair are always
#           adjacent in the PE stream (disjoint row groups -> concurrent).
#           exp groups of 3 tiles run across pairs (a pair may span groups).
#   filler: after each pair, flush 2-4 lagged attn-V units + pump bg items.
#           attn-V lags exp by >=1 group so the PE never waits on ScalarE.
#   qc boundaries: no synchronous drain; blocks retire lazily, outproj(qc-1)
#           items join bg once qc-1's last norm is emitted.
#   PSUM: score groups 2x3 banks + attn-V accum 1 + bg 1 = 8.
#
# The V tiles carry the key mask folded in, plus the mask itself as a 65th
# column per head so the attention-V matmul also produces the softmax
# denominators (row 64).  Normalization: reciprocal of row 64, gpsimd
# partition broadcast, one DVE multiply per head.
#
# Biases: bq/bk added on device (folded into PSUM eviction). bv and bo are
# linear post-softmax terms: since softmax rows sum to one,
# (attn@V + bv)@Wo^T + bo == attn@V@Wo^T + (bv@Wo^T + bo), added on host.

from collections import deque
from contextlib import ExitStack

import numpy as np
import ml_dtypes

import concourse.bass as bass  # noqa: F401  (AP types come via handles)
import concourse.tile as tile
from concourse import bacc, mybir
from concourse.bass_utils import run_bass_kernel_spmd

P = 128
S = 2048          # sequence length
D = 1024          # model dim
E = 512           # per-core head dims (8 heads x 64)
NH = 8            # heads per core
NDCH = D // P     # 8 contraction chunks for projections
NST = S // P      # 16 s-tiles (key chunks)
NSC = 4           # s chunks of 512
NET = E // P      # 4 e-tiles of the local head dims
NHP = NH // 2     # 4 head pairs
NKC = NST         # 16 key chunks of 128
NQC = 4           # query chunks of 512
QCW = S // NQC    # 512
VW = 65           # V columns per head incl. mask column

F32 = mybir.dt.float32
BF16 = mybir.dt.bfloat16
AF = mybir.ActivationFunctionType

EXP_GRP = 3       # scores tiles per exp instruction (3 PSUM banks)


def _mm(nc, out, lhsT, rhs, start, stop):
    nc.tensor.matmul(out, lhsT, rhs, start=start, stop=stop)


def _build_program():
    nc = bacc.Bacc(
        "TRN2",
        debug=False,
        target_bir_lowering=False,
        enable_partition_id=False,
    )

    xT = nc.dram_tensor("xT", [D, S], BF16, kind="ExternalInput").ap()
    wqT = nc.dram_tensor("wqT", [D, E], BF16, kind="ExternalInput").ap()
    wkT = nc.dram_tensor("wkT", [D, E], BF16, kind="ExternalInput").ap()
    wvT = nc.dram_tensor("wvT", [D, E], BF16, kind="ExternalInput").ap()
    woT = nc.dram_tensor("woT", [E, D], BF16, kind="ExternalInput").ap()
    bq_t = nc.dram_tensor("bq_t", [P, NET], F32, kind="ExternalInput").ap()
    bk_t = nc.dram_tensor("bk_t", [P, NET], F32, kind="ExternalInput").ap()
    mk_t = nc.dram_tensor("mk_t", [P, NST], F32, kind="ExternalInput").ap()
    mask8 = nc.dram_tensor("mask8", [NST, P, NH], BF16, kind="ExternalInput").ap()
    fT = nc.dram_tensor("fT", [D, S], F32, kind="ExternalOutput").ap()

    with tile.TileContext(nc) as tc, ExitStack() as ctx:
        pers = ctx.enter_context(tc.tile_pool(name="pers", bufs=1))

        KT = [pers.tile([P, S], BF16, name=f"KT{j}", tag=f"KT{j}") for j in range(NET)]
        QT = [pers.tile([P, S], BF16, name=f"QT{j}", tag=f"QT{j}") for j in range(NET)]
        Vg = [
            pers.tile([P, NH * VW], BF16, name=f"Vg{t}", tag=f"Vg{t}")
            for t in range(NST)
        ]
        xs = [
            [pers.tile([P, QCW], BF16, name=f"x{d}_{sc}", tag=f"x{d}_{sc}")
             for sc in range(NSC)]
            for d in range(NDCH)
        ]
        qw = [pers.tile([P, E], BF16, name=f"qw{d}", tag=f"qw{d}") for d in range(NDCH)]
        wo = [pers.tile([P, D], BF16, name=f"wo{c}", tag=f"wo{c}") for c in range(NET)]
        bq_sb = pers.tile([P, NET], F32, name="bq_sb", tag="bq_sb")
        bk_sb = pers.tile([P, NET], F32, name="bk_sb", tag="bk_sb")
        mk_sb = pers.tile([P, NST], F32, name="mk_sb", tag="mk_sb")

        wpool = ctx.enter_context(tc.tile_pool(name="wpool", bufs=1))
        kw = [wpool.tile([P, E], BF16, name=f"kw{d}", tag=f"kw{d}") for d in range(NDCH)]
        vw = [wpool.tile([P, E], BF16, name=f"vw{d}", tag=f"vw{d}") for d in range(NDCH)]
        wu = wpool.tile([P, P], BF16, name="wu", tag="wu")
        # warmup weights memset FIRST on the gpsimd queue so the PE warmup
        # burst can start as soon as the engines come up.
        nc.gpsimd.memset(wu[:], 0.0)

        # DMA order = need order.  First consumers: kw+x(sc0) (K sc0 proj),
        # then qw (Q qc0), x(sc1) (K sc1), then vw / x(sc2,sc3) / wo.
        # Three queues so the first-needed ~3MB isn't serialized behind
        # late-needed data on any single queue.
        for d in range(NDCH):
            nc.sync.dma_start(kw[d][:], wkT[d * P : (d + 1) * P, :])
        for d in range(NDCH):
            nc.scalar.dma_start(qw[d][:], wqT[d * P : (d + 1) * P, :])
        for d in range(NDCH):
            nc.gpsimd.dma_start(xs[d][0][:], xT[d * P : (d + 1) * P, 0:QCW])
        for d in range(NDCH):
            nc.sync.dma_start(xs[d][1][:], xT[d * P : (d + 1) * P, QCW : 2 * QCW])
        nc.gpsimd.dma_start(bq_sb[:], bq_t)
        nc.gpsimd.dma_start(bk_sb[:], bk_t)
        nc.gpsimd.dma_start(mk_sb[:], mk_t)
        for d in range(NDCH):
            nc.scalar.dma_start(vw[d][:], wvT[d * P : (d + 1) * P, :])
        for d in range(NDCH):
            nc.gpsimd.dma_start(
                xs[d][2][:], xT[d * P : (d + 1) * P, 2 * QCW : 3 * QCW]
            )
        for d in range(NDCH):
            nc.sync.dma_start(
                xs[d][3][:], xT[d * P : (d + 1) * P, 3 * QCW : 4 * QCW]
            )
        for c in range(NET):
            nc.scalar.dma_start(wo[c][:], woT[c * P : (c + 1) * P, :])

        # ---------------- ramp ----------------
        with tc.tile_pool(name="rampp", bufs=1, space="PSUM") as rampp:
            # PE warm-up burst: ~5us of dummy matmuls so the HAM clock gate
            # opens (K=8/8) before the real projections start, and the PE
            # isn't idle while the first kw/x tiles stream in.
            wps = rampp.tile([P, 64], F32, name="wps", tag="wps")
            for i in range(100):
                _mm(nc, wps[:64, :], wu[:, (i % 2) * 64 : (i % 2) * 64 + 64],
                    wu[:, 64:128], start=(i == 0), stop=(i == 99))

        with (
            tc.tile_pool(name="spsum", bufs=2, space="PSUM") as spsum,
            tc.tile_pool(name="pvpsum", bufs=1, space="PSUM") as pvpsum,
            tc.tile_pool(name="bgpsum", bufs=1, space="PSUM") as bgpsum,
            tc.tile_pool(name="ptpool", bufs=20) as ptpool,
            tc.tile_pool(name="otpool", bufs=2) as otpool,
            tc.tile_pool(name="npool", bufs=2) as npool,
            tc.tile_pool(name="ostage", bufs=3) as ostage,
        ):
            # --- projection macro emitters (8 MMs each, one bgpsum bank) ---
            def proj_k(sc, j, pool):
                ssl = slice(sc * QCW, (sc + 1) * QCW)
                ps = pool.tile([P, QCW], F32, name="bgps", tag="bgps")
                for d in range(NDCH):
                    _mm(nc, ps[:], kw[d][:, j * P : (j + 1) * P],
                        xs[d][sc][:], start=(d == 0), stop=(d == NDCH - 1))
                nc.vector.tensor_scalar_add(
                    KT[j][:, ssl], ps[:], bk_sb[:, j : j + 1]
                )

            def proj_q(qc, j, pool):
                ps = pool.tile([P, QCW], F32, name="bgps", tag="bgps")
                for d in range(NDCH):
                    _mm(nc, ps[:], qw[d][:, j * P : (j + 1) * P],
                        xs[d][qc][:], start=(d == 0), stop=(d == NDCH - 1))
                nc.vector.tensor_scalar_add(
                    QT[j][:, qc * QCW : (qc + 1) * QCW], ps[:], bq_sb[:, j : j + 1]
                )

            def bg_k(sc, j):
                return lambda: proj_k(sc, j, bgpsum)

            def bg_q(qc, j):
                return lambda: proj_q(qc, j, bgpsum)

            def bg_v(t):
                def emit():
                    ps = bgpsum.tile([P, QCW], F32, name="bgps", tag="bgps")
                    for d in range(NDCH):
                        _mm(nc, ps[:],
                            xs[d][t // 4][:, (t % 4) * P : (t % 4 + 1) * P],
                            vw[d][:], start=(d == 0), stop=(d == NDCH - 1))
                    vdst = Vg[t][:].rearrange("p (h c) -> p h c", c=VW)
                    nc.vector.tensor_scalar_mul(
                        vdst[:, :, 0:64],
                        ps[:].rearrange("p (h c) -> p h c", c=64),
                        mk_sb[:, t : t + 1],
                    )
                    nc.gpsimd.dma_start(vdst[:, :, 64], mask8[t])
                return emit

            def bg_outproj(qc, j, OTs, pool=None):
                qsl = slice(qc * QCW, (qc + 1) * QCW)

                def emit():
                    pl = pool or bgpsum
                    tg = "pv" if pl is pvpsum else "bgps"
                    ps = pl.tile([P, QCW], F32, name="bgps", tag=tg)
                    for hp in range(NHP):
                        _mm(nc, ps[:], wo[hp][:, j * P : (j + 1) * P], OTs[hp][:],
                            start=(hp == 0), stop=(hp == NHP - 1))
                    ot = ostage.tile([P, QCW], F32, name="os", tag="os")
                    nc.vector.tensor_copy(ot[:], ps[:])
                    nc.sync.dma_start(fT[j * P : (j + 1) * P, qsl], ot[:])
                return emit

            # inline minimal ramp: K(sc0,j0), K(sc1,j0), Q(qc0,j0)
            proj_k(0, 0, bgpsum)
            proj_k(1, 0, bgpsum)
            proj_q(0, 0, bgpsum)

            # bg queue: ordered so item i is consumed a few pairs after it
            # can first be pumped (see block loop pacing).
            bg = deque()
            for j in (1, 2, 3):
                bg.append(bg_k(0, j))
                bg.append(bg_k(1, j))
                bg.append(bg_q(0, j))
                bg.append(bg_k(2, j - 1))
                bg.append(bg_k(3, j - 1))
            bg.append(bg_k(2, 3))
            bg.append(bg_k(3, 3))
            for t in range(NST):
                bg.append(bg_v(t))
            for j in range(NET):
                bg.append(bg_q(1, j))

            def pump(n):
                for _ in range(n):
                    if bg:
                        bg.popleft()()

            def norm_head(pv, h, OT):
                # evict PSUM fast, then off-path normalization chain.
                # HW quirks: partition_broadcast reads physical partition 0
                # and single-input DVE copies may shift partition base, so
                # the sums row is copied down to partition 0 first.
                pvs = npool.tile([P, QCW], F32, name=f"pvs{h}", tag=f"pvs{h}")
                rp = npool.tile([P, QCW], F32, name="rp", tag="rp", bufs=1)
                rc = npool.tile([P, QCW], F32, name="rc", tag="rc", bufs=1)
                bc = npool.tile([P, QCW], F32, name=f"bc{h}", tag=f"bc{h}", bufs=1)
                nc.vector.tensor_copy(pvs[0:VW, :], pv[0:VW, :])
                nc.vector.tensor_copy(rp[0:1, :], pvs[64:65, :])
                nc.vector.reciprocal_approx_fast(rc[0:1, :], rp[0:1, :])
                nc.gpsimd.partition_broadcast(bc[0:64, :], rc[0:1, :], channels=64)
                if h == 0:
                    nc.vector.tensor_mul(OT[0:64, :], pvs[0:64, :], bc[0:64, :])
                else:
                    tmB = npool.tile([P, QCW], BF16, name="tmB", tag="tmB")
                    nc.vector.tensor_mul(tmB[0:64, :], pvs[0:64, :], bc[0:64, :])
                    nc.sync.dma_start(OT[64:128, :], tmB[0:64, :])

            # --- global exp-group stream across all blocks ---
            grp = {"st": None, "pt": None, "cnt": 0, "emitted": 0}

            def add_score_unit(blk, hp, h, kc, qsl):
                i = grp["cnt"]
                if i == 0:
                    grp["st"] = spsum.tile([P, QCW * EXP_GRP], F32, name="st", tag="st")
                    grp["pt"] = ptpool.tile(
                        [P, QCW * EXP_GRP], BF16, name="pt", tag="pt"
                    )
                lo = h * 64
                _mm(
                    nc,
                    grp["st"][:, i * QCW : (i + 1) * QCW],
                    KT[hp][lo : lo + 64, kc * P : (kc + 1) * P],
                    QT[hp][lo : lo + 64, qsl],
                    start=True,
                    stop=True,
                )
                blk["q"][h].append((kc, i, grp["pt"]))
                backlog["n"] += 1
                grp["cnt"] += 1
                if grp["cnt"] == EXP_GRP:
                    close_group()

            def close_group():
                n = grp["cnt"]
                if n == 0:
                    return
                nw = n * QCW
                nc.scalar.activation(
                    grp["pt"][:, :nw], grp["st"][:, :nw], AF.Exp, scale=0.125
                )
                grp["cnt"] = 0
                grp["emitted"] += 1

            # Lagged attn-V: units are consumed h-MAJOR per head pair so a
            # single PSUM bank works: all of h0's 16 kc accumulate and
            # normalize, then h1's.  blocks: one per (qc, hp):
            # {hp, qc, OT, q: per-h unit deques, closed} — unit = (kc, i, pt).
            blocks = deque()
            pv_state = {"pv": None, "key": None}
            backlog = {"n": 0}
            qc_blocks_left = [NHP] * NQC

            def flush_unit():
                while blocks:
                    b = blocks[0]
                    if b["q"][0]:
                        h = 0
                    elif b["closed"] and b["q"][1]:
                        h = 1
                    elif b["closed"]:
                        qc_blocks_left[b["qc"]] -= 1
                        blocks.popleft()
                        continue
                    else:
                        return False  # h0 starved until more pairs land
                    kc, i, pt = b["q"][h].popleft()
                    backlog["n"] -= 1
                    hp = b["hp"]
                    if pv_state["key"] != (id(b), h):
                        pv_state["pv"] = pvpsum.tile([P, QCW], F32, name="pv", tag="pv")
                        pv_state["key"] = (id(b), h)
                    pv = pv_state["pv"]
                    hh = hp * 2 + h
                    _mm(
                        nc,
                        pv[0:VW, :],
                        Vg[kc][:, hh * VW : (hh + 1) * VW],
                        pt[:, i * QCW : (i + 1) * QCW],
                        start=(kc == 0),
                        stop=(kc == NKC - 1),
                    )
                    if kc == NKC - 1:
                        norm_head(pv, h, b["OT"])
                    return True
                return False

            all_OTs = []
            out_appended = [False] * NQC
            pair_ctr = {"n": 0}

            def maybe_append_outproj():
                # outproj(q) becomes available once q's blocks all retired
                # (their norms are emitted).  Append strictly in qc order.
                for q_ in range(NQC - 1):
                    if out_appended[q_]:
                        continue
                    if q_ < len(all_OTs) and qc_blocks_left[q_] == 0:
                        out_appended[q_] = True
                        for j in range(D // P):
                            bg.append(bg_outproj(q_, j, all_OTs[q_]))
                    else:
                        break

            for qc in range(NQC):
                qsl = slice(qc * QCW, (qc + 1) * QCW)
                OTs = [
                    otpool.tile([P, QCW], BF16, name=f"ot{hp}", tag=f"ot{hp}")
                    for hp in range(NHP)
                ]
                all_OTs.append(OTs)
                if 1 <= qc <= NQC - 2:
                    for j in range(NET):
                        bg.append(bg_q(qc + 1, j))

                for hp in range(NHP):
                    defer_all = qc == 0 and hp == 0
                    blk = {
                        "hp": hp, "qc": qc, "OT": OTs[hp],
                        "q": (deque(), deque()), "closed": False,
                    }
                    blocks.append(blk)
                    for kc in range(NKC):
                        add_score_unit(blk, hp, 0, kc, qsl)
                        add_score_unit(blk, hp, 1, kc, qsl)
                        maybe_append_outproj()
                        if defer_all:
                            # projections stream as bg under the first exp
                            # groups; attn-V for this block drains from hp1.
                            pump(2)
                        else:
                            # beat = 3 pairs (= 2 exp groups): score pairs
                            # run back-to-back (no 64/128-row mode switches
                            # inside the run), then one filler burst of
                            # attn-V units + at most one bg macro.
                            pair_ctr["n"] += 1
                            if pair_ctr["n"] % 3 == 0:
                                n = backlog["n"]
                                cap = 12 if n > 30 else (9 if n > 18 else 6)
                                flushed = 0
                                while flushed < cap and flush_unit():
                                    flushed += 1
                                pump(1)
                    blk["closed"] = True

            close_group()
            # tail: drain remaining attn-V (+ any bg) then final outproj,
            # alternating between the two free PSUM pools so the evictions
            # pipeline.
            while flush_unit():
                pump(1)
            maybe_append_outproj()
            pump(len(bg))
            for q_ in range(NQC - 1):
                if not out_appended[q_]:
                    out_appended[q_] = True
                    for j in range(D // P):
                        bg_outproj(q_, j, all_OTs[q_])()
            for j in range(D // P):
                bg_outproj(
                    NQC - 1, j, all_OTs[NQC - 1], pvpsum if j % 2 else bgpsum
                )()

    nc.compile()
    return nc


_PROGRAM = None


def _get_program():
    global _PROGRAM
    if _PROGRAM is None:
        _PROGRAM = _build_program()
    return _PROGRAM


def make_in_maps(x, mask, Wq, Wk, Wv, Wo, bq, bk):
    """Per-core input dicts. Core c: batch c//2, head-group c%2."""
    bf = ml_dtypes.bfloat16
    WqT = np.ascontiguousarray(Wq.T.astype(bf))
    WkT = np.ascontiguousarray(Wk.T.astype(bf))
    WvT = np.ascontiguousarray(Wv.T.astype(bf))
    WoT = np.ascontiguousarray(Wo.T.astype(np.float32))  # [d, e]
    in_maps = []
    for c in range(8):
        b, g = divmod(c, 2)
        esl = slice(g * E, (g + 1) * E)
        m = mask[b].astype(np.float32)
        mk = np.ascontiguousarray(m.reshape(NST, P).T)
        m8 = np.ascontiguousarray(
            np.repeat(m.reshape(NST, P, 1), NH, axis=2).astype(bf)
        )
        in_maps.append(
            {
                "xT": np.ascontiguousarray(x[b].T.astype(bf)),
                "wqT": np.ascontiguousarray(WqT[:, esl]),
                "wkT": np.ascontiguousarray(WkT[:, esl]),
                "wvT": np.ascontiguousarray(WvT[:, esl]),
                "woT": np.ascontiguousarray(WoT[esl, :].astype(bf)),
                "bq_t": np.ascontiguousarray(bq[esl].reshape(NET, P).T.astype(np.float32)),
                "bk_t": np.ascontiguousarray(bk[esl].reshape(NET, P).T.astype(np.float32)),
                "mk_t": mk,
                "mask8": m8,
            }
        )
    return in_maps


def kernel(**inputs):
    x = np.asarray(inputs["x"], dtype=np.float32)
    mask = np.asarray(inputs["mask"])
    Wq = np.asarray(inputs["Wq"], dtype=np.float32)
    Wk = np.asarray(inputs["Wk"], dtype=np.float32)
    Wv = np.asarray(inputs["Wv"], dtype=np.float32)
    Wo = np.asarray(inputs["Wo"], dtype=np.float32)
    bq = np.asarray(inputs["bq"], dtype=np.float32)
    bk = np.asarray(inputs["bk"], dtype=np.float32)
    bv = np.asarray(inputs["bv"], dtype=np.float32)
    bo = np.asarray(inputs["bo"], dtype=np.float32)

    nc = _get_program()
    in_maps = make_in_maps(x, mask, Wq, Wk, Wv, Wo, bq, bk)

    res = run_bass_kernel_spmd(nc, in_maps, core_ids=list(range(8)))

    WoT = Wo.T  # [d, e]
    extra = (bv @ WoT + bo).astype(np.float32)  # [D]
    out = np.empty((4, S, D), dtype=np.float32)
    for b in range(4):
        acc = res.results[2 * b]["fT"] + res.results[2 * b + 1]["fT"]  # [D, S]
        out[b] = acc.T + extra[None, :]
    return out


# revision 22
# speedup vs baseline: 1.0832x; 1.0832x over previous
# Multi-head attention kernel for Trainium2 (Bass/Tile), 8-core SPMD.
#
# Problem: B=4, S=2048, D=1024, H=16 heads, d_k=64 (fp32 in/out).
#
# Sharding: core c = (batch b, head-group g) with b = c//2, g = c%2.
# Each core computes 8 heads of one batch entirely on-device and emits the
# partial final projection (out_heads @ Wo_slice^T) over the full model dim.
# Host sums the two partial outputs per batch and adds the linear bias terms.
#
# Engine budget per core (bf16, warm 2.4 GHz):
#   scores   512 MMs (64-contr, h0/h1 row-group PAIRED)  ~55-65us
#   attn-V   512 MMs (128-contr, N=512)                  ~110us
#   proj     K/Q/V 384 MMs + outproj 128 MMs             ~111us
#   exp      171 ACTIVATE groups of FD=1536              ~262us ScalarE
# PE ~285us busy vs ScalarE ~262us -> PE is critical; the schedule keeps
# the PE dense and starts the exp stream as early as possible (~11us).
#
# Emission structure (engine order == emission order):
#   ramp:   warmup burst (HAM un-throttle) while kw/qw/x(sc0,sc1) stream in;
#           then ONLY K(sc0,j0), K(sc1,j0), Q(qc0,j0) inline -> first score
#           pair ~24 MMs in.  All other projections become bg items, ordered
#           so each is pumped a few pairs before its first consumer.
#   pairs:  per (qc,hp) block, 16 kc-pairs; the two MMs of a pair are always
#           adjacent in the PE stream (disjoint row groups -> concurrent).
#           exp groups of 3 tiles run across pairs (a pair may span groups).
#   filler: after each pair, flush 2-4 lagged attn-V units + pump bg items.
#           attn-V lags exp by >=1 group so the PE never waits on ScalarE.
#   qc boundaries: no synchronous drain; blocks retire lazily, outproj(qc-1)
#           items join bg once qc-1's last norm is emitted.
#   PSUM: score groups 2x3 banks + attn-V accum 1 + bg 1 = 8.
#
# The V tiles carry the key mask folded in, plus the mask itself as a 65th
# column per head so the attention-V matmul also produces the softmax
# denominators (row 64).  Normalization: reciprocal of row 64, gpsimd
# partition broadcast, one DVE multiply per head.
#
# Biases: bq/bk added on device (folded into PSUM eviction). bv and bo are
# linear post-softmax terms: since softmax rows sum to one,
# (attn@V + bv)@Wo^T + bo == attn@V@Wo^T + (bv@Wo^T + bo), added on host.

from collections import deque
from contextlib import ExitStack

import numpy as np
import ml_dtypes

import concourse.bass as bass  # noqa: F401  (AP types come via handles)
import concourse.tile as tile
from concourse import bacc, mybir
from concourse.bass_utils import run_bass_kernel_spmd

P = 128
S = 2048          # sequence length
D = 1024          # model dim
E = 512           # per-core head dims (8 heads x 64)
NH = 8            # heads per core
NDCH = D // P     # 8 contraction chunks for projections
NST = S // P      # 16 s-tiles (key chunks)
NSC = 4           # s chunks of 512
NET = E // P      # 4 e-tiles of the local head dims
NHP = NH // 2     # 4 head pairs
NKC = NST         # 16 key chunks of 128
NQC = 4           # query chunks of 512
QCW = S // NQC    # 512
VW = 65           # V columns per head incl. mask column

F32 = mybir.dt.float32
BF16 = mybir.dt.bfloat16
AF = mybir.ActivationFunctionType

EXP_GRP = 3       # scores tiles per exp instruction (3 PSUM banks)


def _mm(nc, out, lhsT, rhs, start, stop):
    nc.tensor.matmul(out, lhsT, rhs, start=start, stop=stop)


def _build_program():
    nc = bacc.Bacc(
        "TRN2",
        debug=False,
        target_bir_lowering=False,
        enable_partition_id=False,
    )

    xT = nc.dram_tensor("xT", [D, S], BF16, kind="ExternalInput").ap()
    wqT = nc.dram_tensor("wqT", [D, E], BF16, kind="ExternalInput").ap()
    wkT = nc.dram_tensor("wkT", [D, E], BF16, kind="ExternalInput").ap()
    wvT = nc.dram_tensor("wvT", [D, E], BF16, kind="ExternalInput").ap()
    woT = nc.dram_tensor("woT", [E, D], BF16, kind="ExternalInput").ap()
    bq_t = nc.dram_tensor("bq_t", [P, NET], F32, kind="ExternalInput").ap()
    bk_t = nc.dram_tensor("bk_t", [P, NET], F32, kind="ExternalInput").ap()
    mk_t = nc.dram_tensor("mk_t", [P, NST], F32, kind="ExternalInput").ap()
    mask8 = nc.dram_tensor("mask8", [NST, P, NH], BF16, kind="ExternalInput").ap()
    fT = nc.dram_tensor("fT", [D, S], F32, kind="ExternalOutput").ap()

    with tile.TileContext(nc) as tc, ExitStack() as ctx:
        pers = ctx.enter_context(tc.tile_pool(name="pers", bufs=1))

        KT = [pers.tile([P, S], BF16, name=f"KT{j}", tag=f"KT{j}") for j in range(NET)]
        QT = [pers.tile([P, S], BF16, name=f"QT{j}", tag=f"QT{j}") for j in range(NET)]
        Vg = [
            pers.tile([P, NH * VW], BF16, name=f"Vg{t}", tag=f"Vg{t}")
            for t in range(NST)
        ]
        xs = [
            pers.tile([P, S], BF16, name=f"x{d}", tag=f"x{d}")
            for d in range(NDCH)
        ]
        qw = [pers.tile([P, E], BF16, name=f"qw{d}", tag=f"qw{d}") for d in range(NDCH)]
        wo = [pers.tile([P, D], BF16, name=f"wo{c}", tag=f"wo{c}") for c in range(NET)]
        bq_sb = pers.tile([P, NET], F32, name="bq_sb", tag="bq_sb")
        bk_sb = pers.tile([P, NET], F32, name="bk_sb", tag="bk_sb")
        mk_sb = pers.tile([P, NST], F32, name="mk_sb", tag="mk_sb")

        wpool = ctx.enter_context(tc.tile_pool(name="wpool", bufs=1))
        kw = [wpool.tile([P, E], BF16, name=f"kw{d}", tag=f"kw{d}") for d in range(NDCH)]
        vw = [wpool.tile([P, E], BF16, name=f"vw{d}", tag=f"vw{d}") for d in range(NDCH)]
        wu = wpool.tile([P, P], BF16, name="wu", tag="wu")
        # warmup weights memset FIRST on the gpsimd queue so the PE warmup
        # burst can start as soon as the engines come up.
        nc.gpsimd.memset(wu[:], 0.0)

        # DMA order = need order.  x streams as 8 big 512KB transfers (one
        # per d-chunk, alternating two queues) — big transfers amortize the
        # per-dma_start fixed cost so the ramp isn't descriptor-paced.
        # Weights go on the scalar queue: kw first (K proj), then qw, vw, wo.
        nc.gpsimd.dma_start(bq_sb[:], bq_t)
        nc.gpsimd.dma_start(bk_sb[:], bk_t)
        nc.gpsimd.dma_start(mk_sb[:], mk_t)
        for d in range(NDCH):
            q = nc.sync if d % 2 == 0 else nc.gpsimd
            q.dma_start(xs[d][:], xT[d * P : (d + 1) * P, :])
        for d in range(NDCH):
            nc.scalar.dma_start(kw[d][:], wkT[d * P : (d + 1) * P, :])
        for d in range(NDCH):
            nc.scalar.dma_start(qw[d][:], wqT[d * P : (d + 1) * P, :])
        for d in range(NDCH):
            nc.scalar.dma_start(vw[d][:], wvT[d * P : (d + 1) * P, :])
        for c in range(NET):
            nc.scalar.dma_start(wo[c][:], woT[c * P : (c + 1) * P, :])

        # ---------------- ramp ----------------
        with tc.tile_pool(name="rampp", bufs=1, space="PSUM") as rampp:
            # PE warm-up burst: ~5us of dummy matmuls so the HAM clock gate
            # opens (K=8/8) before the real projections start, and the PE
            # isn't idle while the first kw/x tiles stream in.
            wps = rampp.tile([P, 64], F32, name="wps", tag="wps")
            for i in range(100):
                _mm(nc, wps[:64, :], wu[:, (i % 2) * 64 : (i % 2) * 64 + 64],
                    wu[:, 64:128], start=(i == 0), stop=(i == 99))

        with (
            tc.tile_pool(name="spsum", bufs=2, space="PSUM") as spsum,
            tc.tile_pool(name="pvpsum", bufs=1, space="PSUM") as pvpsum,
            tc.tile_pool(name="bgpsum", bufs=1, space="PSUM") as bgpsum,
            tc.tile_pool(name="ptpool", bufs=20) as ptpool,
            tc.tile_pool(name="otpool", bufs=2) as otpool,
            tc.tile_pool(name="npool", bufs=2) as npool,
            tc.tile_pool(name="ostage", bufs=3) as ostage,
        ):
            # --- projection macro emitters (8 MMs each, one bgpsum bank) ---
            k_done = set()
            q_done = set()

            def proj_k(sc, j, pool):
                ssl = slice(sc * QCW, (sc + 1) * QCW)
                ps = pool.tile([P, QCW], F32, name="bgps", tag="bgps")
                for d in range(NDCH):
                    _mm(nc, ps[:], kw[d][:, j * P : (j + 1) * P],
                        xs[d][:, ssl], start=(d == 0), stop=(d == NDCH - 1))
                nc.vector.tensor_scalar_add(
                    KT[j][:, ssl], ps[:], bk_sb[:, j : j + 1]
                )
                k_done.add((sc, j))

            def proj_q(qc, j, pool):
                qsl_ = slice(qc * QCW, (qc + 1) * QCW)
                ps = pool.tile([P, QCW], F32, name="bgps", tag="bgps")
                for d in range(NDCH):
                    _mm(nc, ps[:], qw[d][:, j * P : (j + 1) * P],
                        xs[d][:, qsl_], start=(d == 0), stop=(d == NDCH - 1))
                nc.vector.tensor_scalar_add(
                    QT[j][:, qsl_], ps[:], bq_sb[:, j : j + 1]
                )
                q_done.add((qc, j))

            def bg_k(sc, j):
                return lambda: proj_k(sc, j, bgpsum)

            def bg_q(qc, j):
                return lambda: proj_q(qc, j, bgpsum)

            def bg_v(t):
                def emit():
                    ps = bgpsum.tile([P, QCW], F32, name="bgps", tag="bgps")
                    for d in range(NDCH):
                        _mm(nc, ps[:],
                            xs[d][:, t * P : (t + 1) * P],
                            vw[d][:], start=(d == 0), stop=(d == NDCH - 1))
                    vdst = Vg[t][:].rearrange("p (h c) -> p h c", c=VW)
                    nc.vector.tensor_scalar_mul(
                        vdst[:, :, 0:64],
                        ps[:].rearrange("p (h c) -> p h c", c=64),
                        mk_sb[:, t : t + 1],
                    )
                    nc.gpsimd.dma_start(vdst[:, :, 64], mask8[t])
                    v_done["n"] = max(v_done["n"], t + 1)
                return emit

            def bg_outproj(qc, j, OTs, pool=None):
                qsl = slice(qc * QCW, (qc + 1) * QCW)

                def emit():
                    pl = pool or bgpsum
                    tg = "pv" if pl is pvpsum else "bgps"
                    ps = pl.tile([P, QCW], F32, name="bgps", tag=tg)
                    for hp in range(NHP):
                        _mm(nc, ps[:], wo[hp][:, j * P : (j + 1) * P], OTs[hp][:],
                            start=(hp == 0), stop=(hp == NHP - 1))
                    ot = ostage.tile([P, QCW], F32, name="os", tag="os")
                    nc.vector.tensor_copy(ot[:], ps[:])
                    nc.sync.dma_start(fT[j * P : (j + 1) * P, qsl], ot[:])
                return emit

            v_done = {"n": 0}

            # inline minimal ramp: K(sc0,j0), K(sc1,j0), Q(qc0,j0)
            proj_k(0, 0, bgpsum)
            proj_k(1, 0, bgpsum)
            proj_q(0, 0, bgpsum)

            # bg queue, V-heavy early: Vg tiles unlock attn-V flushing (the
            # v_done gate) so the qc0 backlog stays within the pt pool.
            # K(sc,j)/Q(j) availability is enforced by the k/q_done gates in
            # the pair loop (pump-inline), so this order is perf-only.
            bg = deque()
            for t in range(4):
                bg.append(bg_v(t))
            bg.append(bg_k(2, 0))
            bg.append(bg_k(3, 0))
            bg.append(bg_v(4))
            bg.append(bg_v(5))
            bg.append(bg_k(0, 1))
            bg.append(bg_k(1, 1))
            bg.append(bg_q(0, 1))
            for t in range(6, 10):
                bg.append(bg_v(t))
            bg.append(bg_k(2, 1))
            bg.append(bg_k(3, 1))
            for t in range(10, 14):
                bg.append(bg_v(t))
            bg.append(bg_k(0, 2))
            bg.append(bg_k(1, 2))
            bg.append(bg_q(0, 2))
            bg.append(bg_v(14))
            bg.append(bg_v(15))
            bg.append(bg_k(2, 2))
            bg.append(bg_k(3, 2))
            bg.append(bg_k(0, 3))
            bg.append(bg_k(1, 3))
            bg.append(bg_q(0, 3))
            bg.append(bg_k(2, 3))
            bg.append(bg_k(3, 3))
            for j in range(NET):
                bg.append(bg_q(1, j))

            def pump(n):
                for _ in range(n):
                    if bg:
                        bg.popleft()()

            def norm_head(pv, h, OT):
                # evict PSUM fast, then off-path normalization chain.
                # HW quirks: partition_broadcast reads physical partition 0
                # and single-input DVE copies may shift partition base, so
                # the sums row is copied down to partition 0 first.
                pvs = npool.tile([P, QCW], F32, name=f"pvs{h}", tag=f"pvs{h}")
                rp = npool.tile([P, QCW], F32, name="rp", tag="rp", bufs=1)
                rc = npool.tile([P, QCW], F32, name="rc", tag="rc", bufs=1)
                bc = npool.tile([P, QCW], F32, name=f"bc{h}", tag=f"bc{h}", bufs=1)
                nc.vector.tensor_copy(pvs[0:VW, :], pv[0:VW, :])
                nc.vector.tensor_copy(rp[0:1, :], pvs[64:65, :])
                nc.vector.reciprocal_approx_fast(rc[0:1, :], rp[0:1, :])
                nc.gpsimd.partition_broadcast(bc[0:64, :], rc[0:1, :], channels=64)
                if h == 0:
                    nc.vector.tensor_mul(OT[0:64, :], pvs[0:64, :], bc[0:64, :])
                else:
                    tmB = npool.tile([P, QCW], BF16, name="tmB", tag="tmB")
                    nc.vector.tensor_mul(tmB[0:64, :], pvs[0:64, :], bc[0:64, :])
                    nc.sync.dma_start(OT[64:128, :], tmB[0:64, :])

            # --- global exp-group stream across all blocks ---
            grp = {"st": None, "pt": None, "cnt": 0, "emitted": 0}

            def add_score_unit(blk, hp, h, kc, qsl):
                i = grp["cnt"]
                if i == 0:
                    grp["st"] = spsum.tile([P, QCW * EXP_GRP], F32, name="st", tag="st")
                    grp["pt"] = ptpool.tile(
                        [P, QCW * EXP_GRP], BF16, name="pt", tag="pt"
                    )
                lo = h * 64
                _mm(
                    nc,
                    grp["st"][:, i * QCW : (i + 1) * QCW],
                    KT[hp][lo : lo + 64, kc * P : (kc + 1) * P],
                    QT[hp][lo : lo + 64, qsl],
                    start=True,
                    stop=True,
                )
                blk["q"][h].append((kc, i, grp["pt"]))
                backlog["n"] += 1
                grp["cnt"] += 1
                if grp["cnt"] == EXP_GRP:
                    close_group()

            def close_group():
                n = grp["cnt"]
                if n == 0:
                    return
                nw = n * QCW
                nc.scalar.activation(
                    grp["pt"][:, :nw], grp["st"][:, :nw], AF.Exp, scale=0.125
                )
                grp["cnt"] = 0
                grp["emitted"] += 1

            # Lagged attn-V: units are consumed h-MAJOR per head pair so a
            # single PSUM bank works: all of h0's 16 kc accumulate and
            # normalize, then h1's.  blocks: one per (qc, hp):
            # {hp, qc, OT, q: per-h unit deques, closed} — unit = (kc, i, pt).
            blocks = deque()
            pv_state = {"pv": None, "key": None}
            backlog = {"n": 0}
            qc_blocks_left = [NHP] * NQC

            def flush_unit():
                while blocks:
                    b = blocks[0]
                    if b["q"][0]:
                        h = 0
                    elif b["closed"] and b["q"][1]:
                        h = 1
                    elif b["closed"]:
                        qc_blocks_left[b["qc"]] -= 1
                        blocks.popleft()
                        continue
                    else:
                        return False  # h0 starved until more pairs land
                    if b["q"][h][0][0] >= v_done["n"]:
                        return False  # Vg for this kc not emitted yet
                    kc, i, pt = b["q"][h].popleft()
                    backlog["n"] -= 1
                    hp = b["hp"]
                    if pv_state["key"] != (id(b), h):
                        pv_state["pv"] = pvpsum.tile([P, QCW], F32, name="pv", tag="pv")
                        pv_state["key"] = (id(b), h)
                    pv = pv_state["pv"]
                    hh = hp * 2 + h
                    _mm(
                        nc,
                        pv[0:VW, :],
                        Vg[kc][:, hh * VW : (hh + 1) * VW],
                        pt[:, i * QCW : (i + 1) * QCW],
                        start=(kc == 0),
                        stop=(kc == NKC - 1),
                    )
                    if kc == NKC - 1:
                        norm_head(pv, h, b["OT"])
                    return True
                return False

            all_OTs = []
            out_appended = [False] * NQC
            pair_ctr = {"n": 0}

            def maybe_append_outproj():
                # outproj(q) becomes available once q's blocks all retired
                # (their norms are emitted).  Append strictly in qc order.
                for q_ in range(NQC - 1):
                    if out_appended[q_]:
                        continue
                    if q_ < len(all_OTs) and qc_blocks_left[q_] == 0:
                        out_appended[q_] = True
                        for j in range(D // P):
                            bg.append(bg_outproj(q_, j, all_OTs[q_]))
                    else:
                        break

            for qc in range(NQC):
                qsl = slice(qc * QCW, (qc + 1) * QCW)
                OTs = [
                    otpool.tile([P, QCW], BF16, name=f"ot{hp}", tag=f"ot{hp}")
                    for hp in range(NHP)
                ]
                all_OTs.append(OTs)
                if 1 <= qc <= NQC - 2:
                    for j in range(NET):
                        bg.append(bg_q(qc + 1, j))

                for hp in range(NHP):
                    blk = {
                        "hp": hp, "qc": qc, "OT": OTs[hp],
                        "q": (deque(), deque()), "closed": False,
                    }
                    blocks.append(blk)
                    for kc in range(NKC):
                        # hard emission-order gate: the projections feeding
                        # this pair must be emitted before the score reads.
                        while (kc // 4, hp) not in k_done or (qc, hp) not in q_done:
                            assert bg, "bg exhausted before projections done"
                            pump(1)
                        add_score_unit(blk, hp, 0, kc, qsl)
                        add_score_unit(blk, hp, 1, kc, qsl)
                        maybe_append_outproj()
                        # beat = 3 pairs (= 2 exp groups): score pairs run
                        # back-to-back (no 64/128-row mode switches inside
                        # the run), then one filler burst of attn-V units
                        # plus bg macros (2/beat while qc0's projection and
                        # V work is outstanding, 1/beat after).
                        pair_ctr["n"] += 1
                        if pair_ctr["n"] % 3 == 0:
                            n = backlog["n"]
                            cap = 12 if n > 30 else (9 if n > 18 else 6)
                            flushed = 0
                            while flushed < cap and flush_unit():
                                flushed += 1
                            pump(2 if qc == 0 else 1)
                    blk["closed"] = True

            close_group()
            # tail: drain remaining attn-V interleaved with bg (outproj of
            # earlier qcs), then the final outproj alternating between the
            # two free PSUM pools so the evictions pipeline.
            nflush = 0
            while blocks:
                if flush_unit():
                    nflush += 1
                    maybe_append_outproj()
                    if nflush % 4 == 0 and bg:
                        pump(1)
                else:
                    if not blocks:
                        break
                    assert bg, "tail drain stuck"
                    pump(1)
            maybe_append_outproj()
            pump(len(bg))
            for q_ in range(NQC - 1):
                if not out_appended[q_]:
                    out_appended[q_] = True
                    for j in range(D // P):
                        bg_outproj(q_, j, all_OTs[q_])()
            for j in range(D // P):
                bg_outproj(
                    NQC - 1, j, all_OTs[NQC - 1], pvpsum if j % 2 else bgpsum
                )()

    nc.compile()
    return nc


_PROGRAM = None


def _get_program():
    global _PROGRAM
    if _PROGRAM is None:
        _PROGRAM = _build_program()
    return _PROGRAM


def make_in_maps(x, mask, Wq, Wk, Wv, Wo, bq, bk):
    """Per-core input dicts. Core c: batch c//2, head-group c%2."""
    bf = ml_dtypes.bfloat16
    WqT = np.ascontiguousarray(Wq.T.astype(bf))
    WkT = np.ascontiguousarray(Wk.T.astype(bf))
    WvT = np.ascontiguousarray(Wv.T.astype(bf))
    WoT = np.ascontiguousarray(Wo.T.astype(np.float32))  # [d, e]
    in_maps = []
    for c in range(8):
        b, g = divmod(c, 2)
        esl = slice(g * E, (g + 1) * E)
        m = mask[b].astype(np.float32)
        mk = np.ascontiguousarray(m.reshape(NST, P).T)
        m8 = np.ascontiguousarray(
            np.repeat(m.reshape(NST, P, 1), NH, axis=2).astype(bf)
        )
        in_maps.append(
            {
                "xT": np.ascontiguousarray(x[b].T.astype(bf)),
                "wqT": np.ascontiguousarray(WqT[:, esl]),
                "wkT": np.ascontiguousarray(WkT[:, esl]),
                "wvT": np.ascontiguousarray(WvT[:, esl]),
                "woT": np.ascontiguousarray(WoT[esl, :].astype(bf)),
                "bq_t": np.ascontiguousarray(bq[esl].reshape(NET, P).T.astype(np.float32)),
                "bk_t": np.ascontiguousarray(bk[esl].reshape(NET, P).T.astype(np.float32)),
                "mk_t": mk,
                "mask8": m8,
            }
        )
    return in_maps


def kernel(**inputs):
    x = np.asarray(inputs["x"], dtype=np.float32)
    mask = np.asarray(inputs["mask"])
    Wq = np.asarray(inputs["Wq"], dtype=np.float32)
    Wk = np.asarray(inputs["Wk"], dtype=np.float32)
    Wv = np.asarray(inputs["Wv"], dtype=np.float32)
    Wo = np.asarray(inputs["Wo"], dtype=np.float32)
    bq = np.asarray(inputs["bq"], dtype=np.float32)
    bk = np.asarray(inputs["bk"], dtype=np.float32)
    bv = np.asarray(inputs["bv"], dtype=np.float32)
    bo = np.asarray(inputs["bo"], dtype=np.float32)

    nc = _get_program()
    in_maps = make_in_maps(x, mask, Wq, Wk, Wv, Wo, bq, bk)

    res = run_bass_kernel_spmd(nc, in_maps, core_ids=list(range(8)))

    WoT = Wo.T  # [d, e]
    extra = (bv @ WoT + bo).astype(np.float32)  # [D]
    out = np.empty((4, S, D), dtype=np.float32)
    for b in range(4):
        acc = res.results[2 * b]["fT"] + res.results[2 * b + 1]["fT"]  # [D, S]
        out[b] = acc.T + extra[None, :]
    return out


# revision 27
# speedup vs baseline: 1.0899x; 1.0062x over previous
# Multi-head attention kernel for Trainium2 (Bass/Tile), 8-core SPMD.
#
# Problem: B=4, S=2048, D=1024, H=16 heads, d_k=64 (fp32 in/out).
#
# Sharding: core c = (batch b, head-group g) with b = c//2, g = c%2.
# Each core computes 8 heads of one batch entirely on-device and emits the
# partial final projection (out_heads @ Wo_slice^T) over the full model dim.
# Host sums the two partial outputs per batch and adds the linear bias terms.
#
# Engine budget per core (bf16, warm 2.4 GHz):
#   scores   512 MMs (64-contr, h0/h1 row-group PAIRED)  ~55-65us
#   attn-V   512 MMs (128-contr, N=512)                  ~110us
#   proj     K/Q/V 384 MMs + outproj 128 MMs             ~111us
#   exp      171 ACTIVATE groups of FD=1536              ~262us ScalarE
# PE ~285us busy vs ScalarE ~262us -> PE is critical; the schedule keeps
# the PE dense and starts the exp stream as early as possible (~11us).
#
# Emission structure (engine order == emission order):
#   ramp:   warmup burst (HAM un-throttle) while kw/qw/x(sc0,sc1) stream in;
#           then ONLY K(sc0,j0), K(sc1,j0), Q(qc0,j0) inline -> first score
#           pair ~24 MMs in.  All other projections become bg items, ordered
#           so each is pumped a few pairs before its first consumer.
#   pairs:  per (qc,hp) block, 16 kc-pairs; the two MMs of a pair are always
#           adjacent in the PE stream (disjoint row groups -> concurrent).
#           exp groups of 3 tiles run across pairs (a pair may span groups).
#   filler: after each pair, flush 2-4 lagged attn-V units + pump bg items.
#           attn-V lags exp by >=1 group so the PE never waits on ScalarE.
#   qc boundaries: no synchronous drain; blocks retire lazily, outproj(qc-1)
#           items join bg once qc-1's last norm is emitted.
#   PSUM: score groups 2x3 banks + attn-V accum 1 + bg 1 = 8.
#
# The V tiles carry the key mask folded in, plus the mask itself as a 65th
# column per head so the attention-V matmul also produces the softmax
# denominators (row 64).  Normalization: reciprocal of row 64, gpsimd
# partition broadcast, one DVE multiply per head.
#
# Biases: bq/bk added on device (folded into PSUM eviction). bv and bo are
# linear post-softmax terms: since softmax rows sum to one,
# (attn@V + bv)@Wo^T + bo == attn@V@Wo^T + (bv@Wo^T + bo), added on host.

from collections import deque
from contextlib import ExitStack

import numpy as np
import ml_dtypes

import concourse.bass as bass  # noqa: F401  (AP types come via handles)
import concourse.tile as tile
from concourse import bacc, mybir
from concourse.bass_utils import run_bass_kernel_spmd

P = 128
S = 2048          # sequence length
D = 1024          # model dim
E = 512           # per-core head dims (8 heads x 64)
NH = 8            # heads per core
NDCH = D // P     # 8 contraction chunks for projections
NST = S // P      # 16 s-tiles (key chunks)
NSC = 4           # s chunks of 512
NET = E // P      # 4 e-tiles of the local head dims
NHP = NH // 2     # 4 head pairs
NKC = NST         # 16 key chunks of 128
NQC = 4           # query chunks of 512
QCW = S // NQC    # 512
VW = 65           # V columns per head incl. mask column

F32 = mybir.dt.float32
BF16 = mybir.dt.bfloat16
AF = mybir.ActivationFunctionType

EXP_GRP = 3       # scores tiles per exp instruction (3 PSUM banks)


def _mm(nc, out, lhsT, rhs, start, stop):
    nc.tensor.matmul(out, lhsT, rhs, start=start, stop=stop)


def _build_program():
    nc = bacc.Bacc(
        "TRN2",
        debug=False,
        target_bir_lowering=False,
        enable_partition_id=False,
    )

    xT = nc.dram_tensor("xT", [D, S], BF16, kind="ExternalInput").ap()
    wqT = nc.dram_tensor("wqT", [D, E], BF16, kind="ExternalInput").ap()
    wkT = nc.dram_tensor("wkT", [D, E], BF16, kind="ExternalInput").ap()
    wvT = nc.dram_tensor("wvT", [D, E], BF16, kind="ExternalInput").ap()
    woT = nc.dram_tensor("woT", [E, D], BF16, kind="ExternalInput").ap()
    bq_t = nc.dram_tensor("bq_t", [P, NET], F32, kind="ExternalInput").ap()
    bk_t = nc.dram_tensor("bk_t", [P, NET], F32, kind="ExternalInput").ap()
    mk_t = nc.dram_tensor("mk_t", [P, NST], F32, kind="ExternalInput").ap()
    mask8 = nc.dram_tensor("mask8", [NST, P, NH], BF16, kind="ExternalInput").ap()
    fT = nc.dram_tensor("fT", [D, S], F32, kind="ExternalOutput").ap()

    with tile.TileContext(nc) as tc, ExitStack() as ctx:
        pers = ctx.enter_context(tc.tile_pool(name="pers", bufs=1))

        KT = [pers.tile([P, S], BF16, name=f"KT{j}", tag=f"KT{j}") for j in range(NET)]
        QT = [pers.tile([P, S], BF16, name=f"QT{j}", tag=f"QT{j}") for j in range(NET)]
        Vg = [
            pers.tile([P, NH * VW], BF16, name=f"Vg{t}", tag=f"Vg{t}")
            for t in range(NST)
        ]
        xs = [
            pers.tile([P, S], BF16, name=f"x{d}", tag=f"x{d}")
            for d in range(NDCH)
        ]
        qw = [pers.tile([P, E], BF16, name=f"qw{d}", tag=f"qw{d}") for d in range(NDCH)]
        wo = [pers.tile([P, D], BF16, name=f"wo{c}", tag=f"wo{c}") for c in range(NET)]
        bq_sb = pers.tile([P, NET], F32, name="bq_sb", tag="bq_sb")
        bk_sb = pers.tile([P, NET], F32, name="bk_sb", tag="bk_sb")
        mk_sb = pers.tile([P, NST], F32, name="mk_sb", tag="mk_sb")

        wpool = ctx.enter_context(tc.tile_pool(name="wpool", bufs=1))
        kw = [wpool.tile([P, E], BF16, name=f"kw{d}", tag=f"kw{d}") for d in range(NDCH)]
        vw = [wpool.tile([P, E], BF16, name=f"vw{d}", tag=f"vw{d}") for d in range(NDCH)]

        # DMA order = need order.  NOTHING goes on the scalar queue — the
        # first ACTIVATE would wait behind every weight DMA's ~0.7us issue
        # slot.  sync: kw interleaved with the x halves the K(sc0/1)+Q(qc0)
        # ramp needs; gpsimd: biases, qw, vw, remaining x halves, wo.
        nc.gpsimd.dma_start(bq_sb[:], bq_t)
        nc.gpsimd.dma_start(bk_sb[:], bk_t)
        nc.gpsimd.dma_start(mk_sb[:], mk_t)
        HS = S // 2
        for d in range(NDCH):
            nc.sync.dma_start(kw[d][:], wkT[d * P : (d + 1) * P, :])
            nc.sync.dma_start(xs[d][:, 0:HS], xT[d * P : (d + 1) * P, 0:HS])
        for d in range(NDCH):
            nc.gpsimd.dma_start(qw[d][:], wqT[d * P : (d + 1) * P, :])
        for d in range(NDCH):
            nc.gpsimd.dma_start(vw[d][:], wvT[d * P : (d + 1) * P, :])
        for d in range(4):
            nc.gpsimd.dma_start(xs[d][:, HS:S], xT[d * P : (d + 1) * P, HS:S])
        for d in range(4, NDCH):
            nc.sync.dma_start(xs[d][:, HS:S], xT[d * P : (d + 1) * P, HS:S])
        for c in range(NET):
            nc.gpsimd.dma_start(wo[c][:], woT[c * P : (c + 1) * P, :])
        # pre-load the ACT exp table (~2.7us) so the first real exp doesn't
        # pay it; reads bq_sb (tiny, after its DMA lands).
        etw = pers.tile([P, 1], F32, name="etw", tag="etw")
        nc.scalar.activation(etw[0:1, :], bq_sb[0:1, 0:1], AF.Exp)

        # (no PE warmup burst: the ramp is DMA-paced for its first ~10us
        # anyway, so the HAM clock gate warms during the first dense beats.)
        with (
            tc.tile_pool(name="spsum", bufs=2, space="PSUM") as spsum,
            tc.tile_pool(name="pvpsum", bufs=1, space="PSUM") as pvpsum,
            tc.tile_pool(name="bgpsum", bufs=1, space="PSUM") as bgpsum,
            tc.tile_pool(name="ptpool", bufs=20) as ptpool,
            tc.tile_pool(name="otpool", bufs=2) as otpool,
            tc.tile_pool(name="npool", bufs=2) as npool,
            tc.tile_pool(name="ostage", bufs=3) as ostage,
        ):
            # --- projection macro emitters (8 MMs each, one bgpsum bank) ---
            k_done = set()
            q_done = set()

            def proj_k(sc, j, pool):
                ssl = slice(sc * QCW, (sc + 1) * QCW)
                ps = pool.tile([P, QCW], F32, name="bgps", tag="bgps")
                for d in range(NDCH):
                    _mm(nc, ps[:], kw[d][:, j * P : (j + 1) * P],
                        xs[d][:, ssl], start=(d == 0), stop=(d == NDCH - 1))
                nc.vector.tensor_scalar_add(
                    KT[j][:, ssl], ps[:], bk_sb[:, j : j + 1]
                )
                k_done.add((sc, j))

            def proj_q(qc, j, pool):
                qsl_ = slice(qc * QCW, (qc + 1) * QCW)
                ps = pool.tile([P, QCW], F32, name="bgps", tag="bgps")
                for d in range(NDCH):
                    _mm(nc, ps[:], qw[d][:, j * P : (j + 1) * P],
                        xs[d][:, qsl_], start=(d == 0), stop=(d == NDCH - 1))
                nc.vector.tensor_scalar_add(
                    QT[j][:, qsl_], ps[:], bq_sb[:, j : j + 1]
                )
                q_done.add((qc, j))

            def bg_k(sc, j):
                return lambda: proj_k(sc, j, bgpsum)

            def bg_q(qc, j):
                return lambda: proj_q(qc, j, bgpsum)

            def bg_v(t):
                def emit():
                    ps = bgpsum.tile([P, QCW], F32, name="bgps", tag="bgps")
                    for d in range(NDCH):
                        _mm(nc, ps[:],
                            xs[d][:, t * P : (t + 1) * P],
                            vw[d][:], start=(d == 0), stop=(d == NDCH - 1))
                    vdst = Vg[t][:].rearrange("p (h c) -> p h c", c=VW)
                    nc.vector.tensor_scalar_mul(
                        vdst[:, :, 0:64],
                        ps[:].rearrange("p (h c) -> p h c", c=64),
                        mk_sb[:, t : t + 1],
                    )
                    nc.gpsimd.dma_start(vdst[:, :, 64], mask8[t])
                    v_done["n"] = max(v_done["n"], t + 1)
                return emit

            def bg_outproj(qc, j, OTs, pool=None):
                qsl = slice(qc * QCW, (qc + 1) * QCW)

                def emit():
                    pl = pool or bgpsum
                    tg = "pv" if pl is pvpsum else "bgps"
                    ps = pl.tile([P, QCW], F32, name="bgps", tag=tg)
                    for hp in range(NHP):
                        _mm(nc, ps[:], wo[hp][:, j * P : (j + 1) * P], OTs[hp][:],
                            start=(hp == 0), stop=(hp == NHP - 1))
                    ot = ostage.tile([P, QCW], F32, name="os", tag="os")
                    nc.vector.tensor_copy(ot[:], ps[:])
                    nc.sync.dma_start(fT[j * P : (j + 1) * P, qsl], ot[:])
                return emit

            v_done = {"n": 0}

            # inline minimal ramp: K(sc0,j0), K(sc1,j0), Q(qc0,j0)
            proj_k(0, 0, bgpsum)
            proj_k(1, 0, bgpsum)
            proj_q(0, 0, bgpsum)

            # bg queue, V-heavy early: Vg tiles unlock attn-V flushing (the
            # v_done gate) so the qc0 backlog stays within the pt pool.
            # K(sc,j)/Q(j) availability is enforced by the k/q_done gates in
            # the pair loop (pump-inline), so this order is perf-only.
            bg = deque()
            for t in range(4):
                bg.append(bg_v(t))
            bg.append(bg_k(2, 0))
            bg.append(bg_k(3, 0))
            bg.append(bg_v(4))
            bg.append(bg_v(5))
            bg.append(bg_k(0, 1))
            bg.append(bg_k(1, 1))
            bg.append(bg_q(0, 1))
            for t in range(6, 10):
                bg.append(bg_v(t))
            bg.append(bg_k(2, 1))
            bg.append(bg_k(3, 1))
            for t in range(10, 14):
                bg.append(bg_v(t))
            bg.append(bg_k(0, 2))
            bg.append(bg_k(1, 2))
            bg.append(bg_q(0, 2))
            bg.append(bg_v(14))
            bg.append(bg_v(15))
            bg.append(bg_k(2, 2))
            bg.append(bg_k(3, 2))
            bg.append(bg_k(0, 3))
            bg.append(bg_k(1, 3))
            bg.append(bg_q(0, 3))
            bg.append(bg_k(2, 3))
            bg.append(bg_k(3, 3))
            for j in range(NET):
                bg.append(bg_q(1, j))

            def pump(n):
                for _ in range(n):
                    if bg:
                        bg.popleft()()

            def norm_head(pv, h, OT):
                # evict PSUM fast, then off-path normalization chain.
                # HW quirks: partition_broadcast reads physical partition 0
                # and single-input DVE copies may shift partition base, so
                # the sums row is copied down to partition 0 first.
                pvs = npool.tile([P, QCW], F32, name=f"pvs{h}", tag=f"pvs{h}")
                rp = npool.tile([P, QCW], F32, name="rp", tag="rp", bufs=1)
                rc = npool.tile([P, QCW], F32, name="rc", tag="rc", bufs=1)
                bc = npool.tile([P, QCW], F32, name=f"bc{h}", tag=f"bc{h}", bufs=1)
                nc.vector.tensor_copy(pvs[0:VW, :], pv[0:VW, :])
                nc.vector.tensor_copy(rp[0:1, :], pvs[64:65, :])
                nc.vector.reciprocal_approx_fast(rc[0:1, :], rp[0:1, :])
                nc.gpsimd.partition_broadcast(bc[0:64, :], rc[0:1, :], channels=64)
                if h == 0:
                    nc.vector.tensor_mul(OT[0:64, :], pvs[0:64, :], bc[0:64, :])
                else:
                    tmB = npool.tile([P, QCW], BF16, name="tmB", tag="tmB")
                    nc.vector.tensor_mul(tmB[0:64, :], pvs[0:64, :], bc[0:64, :])
                    nc.sync.dma_start(OT[64:128, :], tmB[0:64, :])

            # --- global exp-group stream across all blocks ---
            grp = {"st": None, "pt": None, "cnt": 0, "emitted": 0}

            def add_score_unit(blk, hp, h, kc, qsl):
                i = grp["cnt"]
                if i == 0:
                    grp["st"] = spsum.tile([P, QCW * EXP_GRP], F32, name="st", tag="st")
                    grp["pt"] = ptpool.tile(
                        [P, QCW * EXP_GRP], BF16, name="pt", tag="pt"
                    )
                lo = h * 64
                _mm(
                    nc,
                    grp["st"][:, i * QCW : (i + 1) * QCW],
                    KT[hp][lo : lo + 64, kc * P : (kc + 1) * P],
                    QT[hp][lo : lo + 64, qsl],
                    start=True,
                    stop=True,
                )
                blk["q"][h].append((kc, i, grp["pt"]))
                backlog["n"] += 1
                grp["cnt"] += 1
                if grp["cnt"] == EXP_GRP:
                    close_group()

            def close_group():
                n = grp["cnt"]
                if n == 0:
                    return
                nw = n * QCW
                nc.scalar.activation(
                    grp["pt"][:, :nw], grp["st"][:, :nw], AF.Exp, scale=0.125
                )
                grp["cnt"] = 0
                grp["emitted"] += 1

            # Lagged attn-V: units are consumed h-MAJOR per head pair so a
            # single PSUM bank works: all of h0's 16 kc accumulate and
            # normalize, then h1's.  blocks: one per (qc, hp):
            # {hp, qc, OT, q: per-h unit deques, closed} — unit = (kc, i, pt).
            blocks = deque()
            pv_state = {"pv": None, "key": None}
            backlog = {"n": 0}
            qc_blocks_left = [NHP] * NQC

            def flush_unit():
                while blocks:
                    b = blocks[0]
                    if b["q"][0]:
                        h = 0
                    elif b["closed"] and b["q"][1]:
                        h = 1
                    elif b["closed"]:
                        qc_blocks_left[b["qc"]] -= 1
                        blocks.popleft()
                        continue
                    else:
                        return False  # h0 starved until more pairs land
                    if b["q"][h][0][0] >= v_done["n"]:
                        return False  # Vg for this kc not emitted yet
                    kc, i, pt = b["q"][h].popleft()
                    backlog["n"] -= 1
                    hp = b["hp"]
                    if pv_state["key"] != (id(b), h):
                        pv_state["pv"] = pvpsum.tile([P, QCW], F32, name="pv", tag="pv")
                        pv_state["key"] = (id(b), h)
                    pv = pv_state["pv"]
                    hh = hp * 2 + h
                    _mm(
                        nc,
                        pv[0:VW, :],
                        Vg[kc][:, hh * VW : (hh + 1) * VW],
                        pt[:, i * QCW : (i + 1) * QCW],
                        start=(kc == 0),
                        stop=(kc == NKC - 1),
                    )
                    if kc == NKC - 1:
                        norm_head(pv, h, b["OT"])
                    return True
                return False

            all_OTs = []
            out_appended = [False] * NQC
            pair_ctr = {"n": 0}

            def maybe_append_outproj(maxq=NQC - 2):
                # outproj(q) becomes available once q's blocks all retired
                # (their norms are emitted).  Append strictly in qc order.
                # outproj(qc2) is held for the tail (maxq) so the PE has
                # work to chew while the last norm chains run.
                for q_ in range(maxq):
                    if out_appended[q_]:
                        continue
                    if q_ < len(all_OTs) and qc_blocks_left[q_] == 0:
                        out_appended[q_] = True
                        for j in range(D // P):
                            bg.append(bg_outproj(q_, j, all_OTs[q_]))
                    else:
                        break

            for qc in range(NQC):
                qsl = slice(qc * QCW, (qc + 1) * QCW)
                OTs = [
                    otpool.tile([P, QCW], BF16, name=f"ot{hp}", tag=f"ot{hp}")
                    for hp in range(NHP)
                ]
                all_OTs.append(OTs)
                if 1 <= qc <= NQC - 2:
                    for j in range(NET):
                        bg.append(bg_q(qc + 1, j))

                for hp in range(NHP):
                    blk = {
                        "hp": hp, "qc": qc, "OT": OTs[hp],
                        "q": (deque(), deque()), "closed": False,
                    }
                    blocks.append(blk)
                    for kc in range(NKC):
                        # hard emission-order gate: the projections feeding
                        # this pair must be emitted before the score reads.
                        while (kc // 4, hp) not in k_done or (qc, hp) not in q_done:
                            assert bg, "bg exhausted before projections done"
                            pump(1)
                        add_score_unit(blk, hp, 0, kc, qsl)
                        add_score_unit(blk, hp, 1, kc, qsl)
                        maybe_append_outproj()
                        # beat = 3 pairs (= 2 exp groups): score pairs run
                        # back-to-back (no 64/128-row mode switches inside
                        # the run), then one filler burst of attn-V units
                        # plus bg macros (2/beat while qc0's projection and
                        # V work is outstanding, 1/beat after).
                        pair_ctr["n"] += 1
                        if pair_ctr["n"] % 3 == 0:
                            n = backlog["n"]
                            cap = 12 if n > 30 else (9 if n > 18 else 6)
                            flushed = 0
                            while flushed < cap and flush_unit():
                                flushed += 1
                            pump(2 if qc == 0 else 1)
                    blk["closed"] = True

            close_group()
            # tail: drain remaining attn-V interleaved with bg (outproj of
            # earlier qcs), then the final outproj alternating between the
            # two free PSUM pools so the evictions pipeline.
            nflush = 0
            while blocks:
                if flush_unit():
                    nflush += 1
                    maybe_append_outproj(NQC - 1)
                    if nflush % 4 == 0 and bg:
                        pump(1)
                else:
                    if not blocks:
                        break
                    assert bg, "tail drain stuck"
                    pump(1)
            maybe_append_outproj(NQC - 1)
            pump(len(bg))
            for q_ in range(NQC - 1):
                if not out_appended[q_]:
                    out_appended[q_] = True
                    for j in range(D // P):
                        bg_outproj(q_, j, all_OTs[q_])()
            for j in range(D // P):
                bg_outproj(
                    NQC - 1, j, all_OTs[NQC - 1], pvpsum if j % 2 else bgpsum
                )()

    nc.compile()
    return nc


_PROGRAM = None


def _get_program():
    global _PROGRAM
    if _PROGRAM is None:
        _PROGRAM = _build_program()
    return _PROGRAM


def make_in_maps(x, mask, Wq, Wk, Wv, Wo, bq, bk):
    """Per-core input dicts. Core c: batch c//2, head-group c%2."""
    bf = ml_dtypes.bfloat16
    WqT = np.ascontiguousarray(Wq.T.astype(bf))
    WkT = np.ascontiguousarray(Wk.T.astype(bf))
    WvT = np.ascontiguousarray(Wv.T.astype(bf))
    WoT = np.ascontiguousarray(Wo.T.astype(np.float32))  # [d, e]
    in_maps = []
    for c in range(8):
        b, g = divmod(c, 2)
        esl = slice(g * E, (g + 1) * E)
        m = mask[b].astype(np.float32)
        mk = np.ascontiguousarray(m.reshape(NST, P).T)
        m8 = np.ascontiguousarray(
            np.repeat(m.reshape(NST, P, 1), NH, axis=2).astype(bf)
        )
        in_maps.append(
            {
                "xT": np.ascontiguousarray(x[b].T.astype(bf)),
                "wqT": np.ascontiguousarray(WqT[:, esl]),
                "wkT": np.ascontiguousarray(WkT[:, esl]),
                "wvT": np.ascontiguousarray(WvT[:, esl]),
                "woT": np.ascontiguousarray(WoT[esl, :].astype(bf)),
                "bq_t": np.ascontiguousarray(bq[esl].reshape(NET, P).T.astype(np.float32)),
                "bk_t": np.ascontiguousarray(bk[esl].reshape(NET, P).T.astype(np.float32)),
                "mk_t": mk,
                "mask8": m8,
            }
        )
    return in_maps


def kernel(**inputs):
    x = np.asarray(inputs["x"], dtype=np.float32)
    mask = np.asarray(inputs["mask"])
    Wq = np.asarray(inputs["Wq"], dtype=np.float32)
    Wk = np.asarray(inputs["Wk"], dtype=np.float32)
    Wv = np.asarray(inputs["Wv"], dtype=np.float32)
    Wo = np.asarray(inputs["Wo"], dtype=np.float32)
    bq = np.asarray(inputs["bq"], dtype=np.float32)
    bk = np.asarray(inputs["bk"], dtype=np.float32)
    bv = np.asarray(inputs["bv"], dtype=np.float32)
    bo = np.asarray(inputs["bo"], dtype=np.float32)

    nc = _get_program()
    in_maps = make_in_maps(x, mask, Wq, Wk, Wv, Wo, bq, bk)

    res = run_bass_kernel_spmd(nc, in_maps, core_ids=list(range(8)))

    WoT = Wo.T  # [d, e]
    extra = (bv @ WoT + bo).astype(np.float32)  # [D]
    out = np.empty((4, S, D), dtype=np.float32)
    for b in range(4):
        acc = res.results[2 * b]["fT"] + res.results[2 * b + 1]["fT"]  # [D, S]
        out[b] = acc.T + extra[None, :]
    return out


# revision 40
# speedup vs baseline: 1.1098x; 1.0183x over previous
# Multi-head attention kernel for Trainium2 (Bass/Tile), 8-core SPMD.
#
# Problem: B=4, S=2048, D=1024, H=16 heads, d_k=64 (fp32 in/out).
#
# Sharding: core c = (batch b, head-group g) with b = c//2, g = c%2.
# Each core computes 8 heads of one batch entirely on-device and emits the
# partial final projection (out_heads @ Wo_slice^T) over the full model dim.
# Host sums the two partial outputs per batch and adds the linear bias terms.
#
# Engine budget per core (bf16, warm 2.4 GHz):
#   scores   512 MMs (64-contr, h0/h1 row-group PAIRED)  ~55-65us
#   attn-V   512 MMs (128-contr, N=512)                  ~110us
#   proj     K/Q/V 384 MMs + outproj 128 MMs             ~111us
#   exp      171 ACTIVATE groups of FD=1536              ~262us ScalarE
# PE ~285us busy vs ScalarE ~262us -> PE is critical; the schedule keeps
# the PE dense and starts the exp stream as early as possible (~11us).
#
# Emission structure (engine order == emission order):
#   ramp:   warmup burst (HAM un-throttle) while kw/qw/x(sc0,sc1) stream in;
#           then ONLY K(sc0,j0), K(sc1,j0), Q(qc0,j0) inline -> first score
#           pair ~24 MMs in.  All other projections become bg items, ordered
#           so each is pumped a few pairs before its first consumer.
#   pairs:  per (qc,hp) block, 16 kc-pairs; the two MMs of a pair are always
#           adjacent in the PE stream (disjoint row groups -> concurrent).
#           exp groups of 3 tiles run across pairs (a pair may span groups).
#   filler: after each pair, flush 2-4 lagged attn-V units + pump bg items.
#           attn-V lags exp by >=1 group so the PE never waits on ScalarE.
#   qc boundaries: no synchronous drain; blocks retire lazily, outproj(qc-1)
#           items join bg once qc-1's last norm is emitted.
#   PSUM: score groups 2x3 banks + attn-V accum 1 + bg 1 = 8.
#
# The V tiles carry the key mask folded in, plus the mask itself as a 65th
# column per head so the attention-V matmul also produces the softmax
# denominators (row 64).  Normalization: reciprocal of row 64, gpsimd
# partition broadcast, one DVE multiply per head.
#
# Biases: bq/bk added on device (folded into PSUM eviction). bv and bo are
# linear post-softmax terms: since softmax rows sum to one,
# (attn@V + bv)@Wo^T + bo == attn@V@Wo^T + (bv@Wo^T + bo), added on host.

from collections import deque
from contextlib import ExitStack

import numpy as np
import ml_dtypes

import concourse.bass as bass  # noqa: F401  (AP types come via handles)
import concourse.tile as tile
from concourse import bacc, mybir
from concourse.bass_utils import run_bass_kernel_spmd

P = 128
S = 2048          # sequence length
D = 1024          # model dim
E = 512           # per-core head dims (8 heads x 64)
NH = 8            # heads per core
NDCH = D // P     # 8 contraction chunks for projections
NST = S // P      # 16 s-tiles (key chunks)
NSC = 4           # s chunks of 512
NET = E // P      # 4 e-tiles of the local head dims
NHP = NH // 2     # 4 head pairs
NKC = NST         # 16 key chunks of 128
NQC = 4           # query chunks of 512
QCW = S // NQC    # 512
VW = 65           # V columns per head incl. mask column

F32 = mybir.dt.float32
BF16 = mybir.dt.bfloat16
AF = mybir.ActivationFunctionType

EXP_GRP = 3       # scores tiles per exp instruction (3 PSUM banks)


def _mm(nc, out, lhsT, rhs, start, stop):
    nc.tensor.matmul(out, lhsT, rhs, start=start, stop=stop)


def _build_program():
    nc = bacc.Bacc(
        "TRN2",
        debug=False,
        target_bir_lowering=False,
        enable_partition_id=False,
    )

    xT = nc.dram_tensor("xT", [D, S], BF16, kind="ExternalInput").ap()
    wqT = nc.dram_tensor("wqT", [D, E], BF16, kind="ExternalInput").ap()
    wkT = nc.dram_tensor("wkT", [D, E], BF16, kind="ExternalInput").ap()
    wvT = nc.dram_tensor("wvT", [D, E], BF16, kind="ExternalInput").ap()
    woT = nc.dram_tensor("woT", [E, D], BF16, kind="ExternalInput").ap()
    bq_t = nc.dram_tensor("bq_t", [P, NET], F32, kind="ExternalInput").ap()
    bk_t = nc.dram_tensor("bk_t", [P, NET], F32, kind="ExternalInput").ap()
    mk_t = nc.dram_tensor("mk_t", [P, NST], F32, kind="ExternalInput").ap()
    mask8 = nc.dram_tensor("mask8", [NST, P, NH], BF16, kind="ExternalInput").ap()
    fT = nc.dram_tensor("fT", [D, S], F32, kind="ExternalOutput").ap()

    with tile.TileContext(nc) as tc, ExitStack() as ctx:
        pers = ctx.enter_context(tc.tile_pool(name="pers", bufs=1))

        KT = [pers.tile([P, S], BF16, name=f"KT{j}", tag=f"KT{j}") for j in range(NET)]
        QT = [pers.tile([P, S], BF16, name=f"QT{j}", tag=f"QT{j}") for j in range(NET)]
        Vg = [
            pers.tile([P, NH * VW], BF16, name=f"Vg{t}", tag=f"Vg{t}")
            for t in range(NST)
        ]
        xs = [
            pers.tile([P, S], BF16, name=f"x{d}", tag=f"x{d}")
            for d in range(NDCH)
        ]
        qw = [pers.tile([P, E], BF16, name=f"qw{d}", tag=f"qw{d}") for d in range(NDCH)]
        wo = [pers.tile([P, D], BF16, name=f"wo{c}", tag=f"wo{c}") for c in range(NET)]
        bq_sb = pers.tile([P, NET], F32, name="bq_sb", tag="bq_sb")
        bk_sb = pers.tile([P, NET], F32, name="bk_sb", tag="bk_sb")
        mk_sb = pers.tile([P, NST], F32, name="mk_sb", tag="mk_sb")

        wpool = ctx.enter_context(tc.tile_pool(name="wpool", bufs=1))
        kw = [wpool.tile([P, E], BF16, name=f"kw{d}", tag=f"kw{d}") for d in range(NDCH)]
        vw = [wpool.tile([P, E], BF16, name=f"vw{d}", tag=f"vw{d}") for d in range(NDCH)]

        # DMA order = need order.  NOTHING goes on the scalar queue — the
        # first ACTIVATE would wait behind every weight DMA's ~0.7us issue
        # slot.  sync: kw interleaved with the x halves the K(sc0/1)+Q(qc0)
        # ramp needs; gpsimd: biases, qw, vw, remaining x halves, wo.
        nc.gpsimd.dma_start(bq_sb[:], bq_t)
        nc.gpsimd.dma_start(bk_sb[:], bk_t)
        nc.gpsimd.dma_start(mk_sb[:], mk_t)
        HS = S // 2
        for d in range(NDCH):
            q = nc.sync if d % 2 == 0 else nc.gpsimd
            q.dma_start(kw[d][:], wkT[d * P : (d + 1) * P, :])
            q.dma_start(xs[d][:, 0:HS], xT[d * P : (d + 1) * P, 0:HS])
        for d in range(NDCH):
            q = nc.sync if d % 2 == 0 else nc.gpsimd
            q.dma_start(qw[d][:], wqT[d * P : (d + 1) * P, :])
        for d in range(NDCH):
            q = nc.sync if d % 2 == 0 else nc.gpsimd
            q.dma_start(vw[d][:], wvT[d * P : (d + 1) * P, :])
        for d in range(NDCH):
            q = nc.sync if d % 2 == 0 else nc.gpsimd
            q.dma_start(xs[d][:, HS:S], xT[d * P : (d + 1) * P, HS:S])
        for c in range(NET):
            q = nc.sync if c % 2 == 0 else nc.gpsimd
            q.dma_start(wo[c][:], woT[c * P : (c + 1) * P, :])
        # pre-load the ACT exp table (~2.7us) so the first real exp doesn't
        # pay it; reads bq_sb (tiny, after its DMA lands).
        etw = pers.tile([P, 1], F32, name="etw", tag="etw")
        nc.scalar.activation(etw[0:1, :], bq_sb[0:1, 0:1], AF.Exp)

        # (no PE warmup burst: the ramp is DMA-paced for its first ~10us
        # anyway, so the HAM clock gate warms during the first dense beats.)
        with (
            tc.tile_pool(name="pvpsum", bufs=1, space="PSUM") as pvpsum,
            tc.tile_pool(name="bgpsum", bufs=1, space="PSUM") as bgpsum,
            tc.tile_pool(name="ptpool", bufs=19) as ptpool,
            tc.tile_pool(name="otpool", bufs=3) as otpool,
            tc.tile_pool(name="npool", bufs=2) as npool,
            tc.tile_pool(name="ostage", bufs=3) as ostage,
            tc.tile_pool(name="spsum", bufs=2, space="PSUM") as spsum,
        ):
            # --- projection macro emitters (8 MMs each, one bgpsum bank) ---
            k_done = set()
            q_done = set()

            def proj_k(sc, j, pool):
                ssl = slice(sc * QCW, (sc + 1) * QCW)
                ps = pool.tile([P, QCW], F32, name="bgps", tag="bgps")
                for d in range(NDCH):
                    _mm(nc, ps[:], kw[d][:, j * P : (j + 1) * P],
                        xs[d][:, ssl], start=(d == 0), stop=(d == NDCH - 1))
                nc.vector.tensor_scalar_add(
                    KT[j][:, ssl], ps[:], bk_sb[:, j : j + 1]
                )
                k_done.add((sc, j))

            def proj_q(qc, j, pool):
                qsl_ = slice(qc * QCW, (qc + 1) * QCW)
                ps = pool.tile([P, QCW], F32, name="bgps", tag="bgps")
                for d in range(NDCH):
                    _mm(nc, ps[:], qw[d][:, j * P : (j + 1) * P],
                        xs[d][:, qsl_], start=(d == 0), stop=(d == NDCH - 1))
                nc.vector.tensor_scalar_add(
                    QT[j][:, qsl_], ps[:], bq_sb[:, j : j + 1]
                )
                q_done.add((qc, j))

            def bg_k(sc, j):
                return lambda: proj_k(sc, j, bgpsum)

            def bg_q(qc, j):
                return lambda: proj_q(qc, j, bgpsum)

            def bg_v(t):
                def emit():
                    ps = bgpsum.tile([P, QCW], F32, name="bgps", tag="bgps")
                    for d in range(NDCH):
                        _mm(nc, ps[:],
                            xs[d][:, t * P : (t + 1) * P],
                            vw[d][:], start=(d == 0), stop=(d == NDCH - 1))
                    vdst = Vg[t][:].rearrange("p (h c) -> p h c", c=VW)
                    nc.vector.tensor_scalar_mul(
                        vdst[:, :, 0:64],
                        ps[:].rearrange("p (h c) -> p h c", c=64),
                        mk_sb[:, t : t + 1],
                    )
                    nc.gpsimd.dma_start(vdst[:, :, 64], mask8[t])
                    v_done["n"] = max(v_done["n"], t + 1)
                return emit

            def bg_outproj(qc, j, OTs, pool=None):
                qsl = slice(qc * QCW, (qc + 1) * QCW)

                def emit():
                    pl = pool or bgpsum
                    tg = "pv" if pl is pvpsum else "bgps"
                    ps = pl.tile([P, QCW], F32, name="bgps", tag=tg)
                    for hp in range(NHP):
                        _mm(nc, ps[:], wo[hp][:, j * P : (j + 1) * P], OTs[hp][:],
                            start=(hp == 0), stop=(hp == NHP - 1))
                    ot = ostage.tile([P, QCW], F32, name="os", tag="os")
                    nc.vector.tensor_copy(ot[:], ps[:])
                    nc.sync.dma_start(fT[j * P : (j + 1) * P, qsl], ot[:])
                return emit

            v_done = {"n": 0}

            # inline minimal ramp: K(sc0,j0), K(sc1,j0), Q(qc0,j0)
            proj_k(0, 0, bgpsum)
            proj_k(1, 0, bgpsum)
            proj_q(0, 0, bgpsum)

            # bg queue, V-heavy early: Vg tiles unlock attn-V flushing (the
            # v_done gate) so the qc0 backlog stays within the pt pool.
            # K(sc,j)/Q(j) availability is enforced by the k/q_done gates in
            # the pair loop (pump-inline), so this order is perf-only.
            bg = deque()
            for t in range(4):
                bg.append(bg_v(t))
            bg.append(bg_k(2, 0))
            bg.append(bg_k(3, 0))
            bg.append(bg_v(4))
            bg.append(bg_v(5))
            bg.append(bg_k(0, 1))
            bg.append(bg_k(1, 1))
            bg.append(bg_q(0, 1))
            for t in range(6, 10):
                bg.append(bg_v(t))
            bg.append(bg_k(2, 1))
            bg.append(bg_k(3, 1))
            for t in range(10, 14):
                bg.append(bg_v(t))
            bg.append(bg_k(0, 2))
            bg.append(bg_k(1, 2))
            bg.append(bg_q(0, 2))
            bg.append(bg_v(14))
            bg.append(bg_v(15))
            bg.append(bg_k(2, 2))
            bg.append(bg_k(3, 2))
            bg.append(bg_k(0, 3))
            bg.append(bg_k(1, 3))
            bg.append(bg_q(0, 3))
            bg.append(bg_k(2, 3))
            bg.append(bg_k(3, 3))
            for j in range(NET):
                bg.append(bg_q(1, j))

            def pump(n):
                for _ in range(n):
                    if bg:
                        bg.popleft()()

            def norm_head(pv, h, OT):
                # evict PSUM fast, then off-path normalization chain.
                # HW quirks: partition_broadcast reads physical partition 0
                # and single-input DVE copies may shift partition base, so
                # the sums row is copied down to partition 0 first.
                pvs = npool.tile([P, QCW], F32, name=f"pvs{h}", tag=f"pvs{h}")
                rp = npool.tile([P, QCW], F32, name="rp", tag="rp", bufs=1)
                rc = npool.tile([P, QCW], F32, name="rc", tag="rc", bufs=1)
                bc = npool.tile([P, QCW], F32, name=f"bc{h}", tag=f"bc{h}", bufs=1)
                nc.vector.tensor_copy(pvs[0:VW, :], pv[0:VW, :])
                nc.vector.tensor_copy(rp[0:1, :], pvs[64:65, :])
                nc.vector.reciprocal_approx_fast(rc[0:1, :], rp[0:1, :])
                nc.gpsimd.partition_broadcast(bc[0:64, :], rc[0:1, :], channels=64)
                if h == 0:
                    nc.vector.tensor_mul(OT[0:64, :], pvs[0:64, :], bc[0:64, :])
                else:
                    tmB = npool.tile([P, QCW], BF16, name="tmB", tag="tmB")
                    nc.vector.tensor_mul(tmB[0:64, :], pvs[0:64, :], bc[0:64, :])
                    nc.sync.dma_start(OT[64:128, :], tmB[0:64, :])

            # --- global exp-group stream across all blocks ---
            grp = {"st": None, "pt": None, "cnt": 0, "emitted": 0}

            def add_score_unit(blk, hp, h, kc, qsl):
                i = grp["cnt"]
                if i == 0:
                    grp["st"] = spsum.tile([P, QCW * EXP_GRP], F32, name="st", tag="st")
                    grp["pt"] = ptpool.tile(
                        [P, QCW * EXP_GRP], BF16, name="pt", tag="pt"
                    )
                lo = h * 64
                _mm(
                    nc,
                    grp["st"][:, i * QCW : (i + 1) * QCW],
                    KT[hp][lo : lo + 64, kc * P : (kc + 1) * P],
                    QT[hp][lo : lo + 64, qsl],
                    start=True,
                    stop=True,
                )
                blk["q"][h].append((kc, i, grp["pt"], grp["emitted"]))
                backlog["n"] += 1
                grp["cnt"] += 1
                if grp["cnt"] == EXP_GRP:
                    close_group()

            def close_group():
                n = grp["cnt"]
                if n == 0:
                    return
                nw = n * QCW
                nc.scalar.activation(
                    grp["pt"][:, :nw], grp["st"][:, :nw], AF.Exp, scale=0.125
                )
                grp["cnt"] = 0
                grp["emitted"] += 1

            # Lagged attn-V: units are consumed h-MAJOR per head pair so a
            # single PSUM bank works: all of h0's 16 kc accumulate and
            # normalize, then h1's.  blocks: one per (qc, hp):
            # {hp, qc, OT, q: per-h unit deques, closed} — unit = (kc, i, pt).
            blocks = deque()
            pv_state = {"pv": None, "key": None}
            backlog = {"n": 0}
            qc_blocks_left = [NHP] * NQC

            def flush_unit(min_lag=2):
                while blocks:
                    b = blocks[0]
                    ha, hb = b.get("horder", (0, 1))
                    if b["q"][ha] and (ha == 0 or b["closed"]):
                        h = ha
                    elif b["closed"] and b["q"][hb]:
                        h = hb
                    elif b["closed"]:
                        qc_blocks_left[b["qc"]] -= 1
                        blocks.popleft()
                        continue
                    else:
                        return False  # first head starved until more pairs land
                    if b["q"][h][0][0] >= v_done["n"]:
                        return False  # Vg for this kc not emitted yet
                    if grp["emitted"] - b["q"][h][0][3] < min_lag:
                        return False  # exp for this unit too recent — would stall PE
                    kc, i, pt, _ = b["q"][h].popleft()
                    backlog["n"] -= 1
                    hp = b["hp"]
                    if pv_state["key"] != (id(b), h):
                        pv_state["pv"] = pvpsum.tile([P, QCW], F32, name="pv", tag="pv")
                        pv_state["key"] = (id(b), h)
                    pv = pv_state["pv"]
                    hh = hp * 2 + h
                    _mm(
                        nc,
                        pv[0:VW, :],
                        Vg[kc][:, hh * VW : (hh + 1) * VW],
                        pt[:, i * QCW : (i + 1) * QCW],
                        start=(kc == 0),
                        stop=(kc == NKC - 1),
                    )
                    if kc == NKC - 1:
                        norm_head(pv, h, b["OT"])
                    return True
                return False

            all_OTs = []
            out_appended = [False] * NQC
            pair_ctr = {"n": 0}

            def maybe_append_outproj(maxq=NQC - 2):
                # outproj(q) becomes available once q's blocks all retired
                # (their norms are emitted).  Append strictly in qc order.
                # outproj(qc2) is held for the tail (maxq) so the PE has
                # work to chew while the last norm chains run.
                for q_ in range(maxq):
                    if out_appended[q_]:
                        continue
                    if q_ < len(all_OTs) and qc_blocks_left[q_] == 0:
                        out_appended[q_] = True
                        for j in range(D // P):
                            bg.append(bg_outproj(q_, j, all_OTs[q_]))
                    else:
                        break

            for qc in range(NQC):
                qsl = slice(qc * QCW, (qc + 1) * QCW)
                OTs = [
                    otpool.tile([P, QCW], BF16, name=f"ot{hp}", tag=f"ot{hp}")
                    for hp in range(NHP)
                ]
                all_OTs.append(OTs)
                if 1 <= qc <= NQC - 2:
                    for j in range(NET):
                        bg.append(bg_q(qc + 1, j))

                for hp in range(NHP):
                    blk = {
                        "hp": hp, "qc": qc, "OT": OTs[hp],
                        "q": (deque(), deque()), "closed": False,
                    }
                    if qc == NQC - 1 and hp == NHP - 1:
                        # flush h1 first so the very last norm is h0's,
                        # whose OT write is a direct DVE mul (no sbuf-sbuf
                        # DMA on the critical tail chain).
                        blk["horder"] = (1, 0)
                    blocks.append(blk)
                    for kc in range(NKC):
                        # hard emission-order gate: the projections feeding
                        # this pair must be emitted before the score reads.
                        while (kc // 4, hp) not in k_done or (qc, hp) not in q_done:
                            assert bg, "bg exhausted before projections done"
                            pump(1)
                        add_score_unit(blk, hp, 0, kc, qsl)
                        add_score_unit(blk, hp, 1, kc, qsl)
                        # outproj(q) is deferred to qc q+2 (otpool bufs=3
                        # makes the OT slots live that long) so qc3 — which
                        # has no projection work left — still gets PE filler.
                        maybe_append_outproj(max(0, qc - 1))
                        # beat = 3 pairs (= 2 exp groups): score pairs run
                        # back-to-back (no 64/128-row mode switches inside
                        # the run), then a filler burst of attn-V units with
                        # bg macros split around them (adjacent bg macros
                        # would stall ~750ns on the shared bgpsum bank's
                        # eviction).
                        pair_ctr["n"] += 1
                        if pair_ctr["n"] % 3 == 0:
                            n = backlog["n"]
                            cap = 12 if n > 30 else (9 if n > 18 else 6)
                            flushed = 0
                            while flushed < cap // 2 and flush_unit():
                                flushed += 1
                            pump(1)
                            while flushed < cap and flush_unit():
                                flushed += 1
                            if qc == 0:
                                pump(1)
                    blk["closed"] = True

            close_group()
            # tail: the score PSUM banks are free now — grab both spsum
            # slots (their pool-WAR orders them behind the final exp reads)
            # and use the 6 banks to accumulate outproj(qc3) j0..5
            # progressively as each (qc3, hp) block retires, so only j6/j7
            # plus the last hp's partials remain after the final norm.
            stA = spsum.tile([P, QCW * EXP_GRP], F32, name="stA", tag="st")
            stB = spsum.tile([P, QCW * EXP_GRP], F32, name="stB", tag="st")
            NFO = 6
            fo = [
                (stA if j < 3 else stB)[:, (j % 3) * QCW : (j % 3 + 1) * QCW]
                for j in range(NFO)
            ]
            fo_hp = {"n": 0}
            OT3 = all_OTs[NQC - 1]
            q3sl = slice((NQC - 1) * QCW, NQC * QCW)

            def pump_partials():
                while fo_hp["n"] < NHP - qc_blocks_left[NQC - 1]:
                    hp = fo_hp["n"]
                    for j in range(NFO):
                        _mm(nc, fo[j][:], wo[hp][:, j * P : (j + 1) * P],
                            OT3[hp][:], start=(hp == 0), stop=(hp == NHP - 1))
                    fo_hp["n"] += 1

            nflush = 0
            while blocks:
                if flush_unit(min_lag=1):
                    nflush += 1
                    maybe_append_outproj(NQC - 1)
                    pump_partials()
                    if nflush % 4 == 0 and bg:
                        pump(1)
                else:
                    if not blocks:
                        break
                    assert bg, "tail drain stuck"
                    pump(1)
            maybe_append_outproj(NQC - 1)
            pump_partials()
            pump(len(bg))
            for q_ in range(NQC - 1):
                if not out_appended[q_]:
                    out_appended[q_] = True
                    for j in range(D // P):
                        bg_outproj(q_, j, all_OTs[q_])()
            for j in range(NFO):
                ot = ostage.tile([P, QCW], F32, name="os", tag="os")
                nc.vector.tensor_copy(ot[:], fo[j][:])
                nc.sync.dma_start(fT[j * P : (j + 1) * P, q3sl], ot[:])
            for j in range(NFO, D // P):
                bg_outproj(
                    NQC - 1, j, OT3, pvpsum if j % 2 else bgpsum
                )()

    nc.compile()
    return nc


_PROGRAM = None


def _get_program():
    global _PROGRAM
    if _PROGRAM is None:
        _PROGRAM = _build_program()
    return _PROGRAM


def make_in_maps(x, mask, Wq, Wk, Wv, Wo, bq, bk):
    """Per-core input dicts. Core c: batch c//2, head-group c%2."""
    bf = ml_dtypes.bfloat16
    WqT = np.ascontiguousarray(Wq.T.astype(bf))
    WkT = np.ascontiguousarray(Wk.T.astype(bf))
    WvT = np.ascontiguousarray(Wv.T.astype(bf))
    WoT = np.ascontiguousarray(Wo.T.astype(np.float32))  # [d, e]
    in_maps = []
    for c in range(8):
        b, g = divmod(c, 2)
        esl = slice(g * E, (g + 1) * E)
        m = mask[b].astype(np.float32)
        mk = np.ascontiguousarray(m.reshape(NST, P).T)
        m8 = np.ascontiguousarray(
            np.repeat(m.reshape(NST, P, 1), NH, axis=2).astype(bf)
        )
        in_maps.append(
            {
                "xT": np.ascontiguousarray(x[b].T.astype(bf)),
                "wqT": np.ascontiguousarray(WqT[:, esl]),
                "wkT": np.ascontiguousarray(WkT[:, esl]),
                "wvT": np.ascontiguousarray(WvT[:, esl]),
                "woT": np.ascontiguousarray(WoT[esl, :].astype(bf)),
                "bq_t": np.ascontiguousarray(bq[esl].reshape(NET, P).T.astype(np.float32)),
                "bk_t": np.ascontiguousarray(bk[esl].reshape(NET, P).T.astype(np.float32)),
                "mk_t": mk,
                "mask8": m8,
            }
        )
    return in_maps


def kernel(**inputs):
    x = np.asarray(inputs["x"], dtype=np.float32)
    mask = np.asarray(inputs["mask"])
    Wq = np.asarray(inputs["Wq"], dtype=np.float32)
    Wk = np.asarray(inputs["Wk"], dtype=np.float32)
    Wv = np.asarray(inputs["Wv"], dtype=np.float32)
    Wo = np.asarray(inputs["Wo"], dtype=np.float32)
    bq = np.asarray(inputs["bq"], dtype=np.float32)
    bk = np.asarray(inputs["bk"], dtype=np.float32)
    bv = np.asarray(inputs["bv"], dtype=np.float32)
    bo = np.asarray(inputs["bo"], dtype=np.float32)

    nc = _get_program()
    in_maps = make_in_maps(x, mask, Wq, Wk, Wv, Wo, bq, bk)

    res = run_bass_kernel_spmd(nc, in_maps, core_ids=list(range(8)))

    WoT = Wo.T  # [d, e]
    extra = (bv @ WoT + bo).astype(np.float32)  # [D]
    out = np.empty((4, S, D), dtype=np.float32)
    for b in range(4):
        acc = res.results[2 * b]["fT"] + res.results[2 * b + 1]["fT"]  # [D, S]
        out[b] = acc.T + extra[None, :]
    return out
